# revision 1
# baseline (speedup 1.0000x reference)
"""Trainium2 Bass kernel for KANPolyLayer:
    y[b,o] = sum_{i,p} x[b,i]^p * coeffs[o,i,p] + bias[o],  p = 0..4

Math: y = sum_{p=1..4} (x^p) @ C_p^T + (bias + colsum(C_0)), with
C_p = coeffs[:, :, p].  Implemented as 4 accumulated GEMM planes in
float32r (FP22 truncated fp32, full PE rate) with powers computed
on-chip (ScalarE square + VectorE muls).

Per-core schedule: the x^p power slabs ([i, b] layout) are resident in
SBUF; coefficient tiles stream through a small ring.  All 8 output
groups (4 o-tiles x 2 b-halves) accumulate concurrently in 8 PSUM
banks, so each arriving coefficient tile immediately unlocks 8 matmuls
and the PE never waits on the 10 MB coefficient stream.  The p=0
constant column and bias are reduced on-device with small matmuls into
a PSUM column, then applied as a per-partition scalar during the
PSUM->SBUF copy.  The kernel computes yT = [o, b]; host transposes.

Sharding (8 cores): 4 batch groups x 2 out-dim groups.
  core c -> (bg, og) = (c // 2, c % 2)
  per-core x slice:    rows [bg*1024, (bg+1)*1024)   (transposed on host)
  per-core out slice:  cols [og*512, (og+1)*512)
Each core computes a disjoint (512 x 1024) block of yT; host gathers.
"""

from contextlib import ExitStack

import numpy as np

import concourse.bacc as bacc
import concourse.bass as bass
import concourse.mybir as mybir
import concourse.tile as tile
from concourse.bass_utils import run_bass_kernel_spmd

F32 = mybir.dt.float32
F32R = mybir.dt.float32r

B, I, O = 4096, 1024, 1024  # batch, in_dim, out_dim
BW, OW = 4, 2               # batch groups x out-dim groups (8 cores)
BS, OS = B // BW, O // OW   # per-core batch (1024) and out (512)
NK = I // 128               # contraction tiles (8)
NT = OS // 128              # o-tiles (4)
NH = BS // 512              # b-halves (2)

_CACHE: dict = {}


def _build():
    nc = bacc.Bacc("TRN2", target_bir_lowering=False, debug=False, num_devices=8)

    xt = nc.dram_tensor("xt", [I, BS], F32, kind="ExternalInput")      # [i, b]
    ct = nc.dram_tensor("ct", [4, I, OS], F32, kind="ExternalInput")   # [p-1, i, o]
    c0o = nc.dram_tensor("c0o", [OS, I], F32, kind="ExternalInput")    # [o, i]
    biasc = nc.dram_tensor("biasc", [OS, 1], F32, kind="ExternalInput")
    yt = nc.dram_tensor("yt", [OS, BS], F32, kind="ExternalOutput")    # [o, b]

    NTAIL = 2  # trailing k-planes emitted group-contiguous (tail stagger)

    with tile.TileContext(nc) as tc, ExitStack() as ctx:
        cons = ctx.enter_context(tc.tile_pool(name="cons", bufs=1))
        c0pool = ctx.enter_context(tc.tile_pool(name="c0", bufs=4))
        cpool = ctx.enter_context(tc.tile_pool(name="coef", bufs=12))
        ppool = ctx.enter_context(tc.tile_pool(name="pow", bufs=1))
        opool = ctx.enter_context(tc.tile_pool(name="out", bufs=3))
        pspool = ctx.enter_context(
            tc.tile_pool(name="ps", bufs=8, space=bass.MemorySpace.PSUM)
        )

        # 8 concurrent accumulation groups: (o-tile, b-half) -> one PSUM bank
        ps = {}
        for ot in range(NT):
            for h in range(NH):
                ps[(ot, h)] = pspool.tile(
                    [128, 512], F32, tag="ps", name=f"ps_{ot}_{h}"
                )

        # PE warmup: garbage matmuls on a memset tile while the first input
        # DMAs are in flight, so the HAM clock-gate reaches 2.4 GHz before
        # the real stream starts (saves the ~2us cold-start penalty).
        wz = cons.tile([128, 512], F32)
        nc.vector.memset(wz[:], 0.0)
        wr = cons.tile([128, 512], F32R)
        nc.vector.tensor_copy(wr[:], wz[:])
        for w in range(18):
            nc.tensor.matmul(
                ps[(0, 0)][:, 0:256], wr[:, 0:128], wr[:, 0:256], start=True, stop=True,
                skip_group_check=True,
            )

        pows = {}
        cpts = {}
        for k in range(NK):
            tail_k = k >= NK - NTAIL
            # k0: coefficient tile first (smaller -> lands first)
            if k == 0:
                cpt = cpool.tile([128, OS], F32R, tag="cp", name="cpt_0_1")
                nc.sync.dma_start(cpt[:], ct[0, 0:128, :].bitcast(F32R))
                cpts[(0, 1)] = cpt
            # resident power tiles [i=128, b=512] per b-half for this k;
            # separate tiles per half so the first matmuls only wait on
            # half the x DMA bytes
            pk = {}
            for h2 in range(NH):
                x1 = ppool.tile([128, 512], F32R, tag=f"p1_{k}_{h2}",
                                name=f"x1_{k}_{h2}")
                nc.sync.dma_start(
                    x1[:],
                    xt[k * 128:(k + 1) * 128,
                       h2 * 512:(h2 + 1) * 512].bitcast(F32R),
                )
                p2 = ppool.tile([128, 512], F32R, tag=f"p2_{k}_{h2}",
                                name=f"p2_{k}_{h2}")
                p3 = ppool.tile([128, 512], F32R, tag=f"p3_{k}_{h2}",
                                name=f"p3_{k}_{h2}")
                p4 = ppool.tile([128, 512], F32R, tag=f"p4_{k}_{h2}",
                                name=f"p4_{k}_{h2}")
                nc.scalar.square(p2[:], x1[:])
                nc.vector.tensor_mul(p3[:], p2[:], x1[:])
                nc.vector.tensor_mul(p4[:], p2[:], p2[:])
                pk[(1, h2)] = x1
                pk[(2, h2)] = p2
                pk[(3, h2)] = p3
                pk[(4, h2)] = p4
            pows[k] = pk

            for p in range(1, 5):
                if (k, p) not in cpts:
                    cpt = cpool.tile(
                        [128, OS], F32R, tag="cp", name=f"cpt_{k}_{p}"
                    )
                    nc.sync.dma_start(
                        cpt[:], ct[p - 1, k * 128:(k + 1) * 128, :].bitcast(F32R)
                    )
                    cpts[(k, p)] = cpt
                if not tail_k:
                    for ot in range(NT):
                        for h in range(NH):
                            nc.tensor.matmul(
                                ps[(ot, h)],
                                cpts[(k, p)][:, ot * 128:(ot + 1) * 128],
                                pows[k][(p, h)][:],
                                start=(k == 0 and p == 1),
                                stop=False,
                            )

        # bias/C0 inputs stream behind the main inputs (only needed at end):
        # biascol[o-part, ot] = bias[o] + sum_i C0[i, o], DVE-only.
        red = cons.tile([128, NT], F32)
        for ot in range(NT):
            c0s = c0pool.tile([128, I], F32, tag="c0", name=f"c0s_{ot}")
            nc.sync.dma_start(c0s[:], c0o[ot * 128:(ot + 1) * 128, :])
            nc.vector.tensor_reduce(
                red[:, ot:ot + 1], c0s[:], mybir.AxisListType.X, mybir.AluOpType.add
            )
        biasc_sb = cons.tile([128, NT], F32)
        for ot in range(NT):
            nc.sync.dma_start(
                biasc_sb[:, ot:ot + 1], biasc[ot * 128:(ot + 1) * 128, :]
            )
        biascol = cons.tile([128, NT], F32)
        nc.vector.tensor_add(biascol[:], red[:], biasc_sb[:])

        # trailing k-planes group-contiguous: each group finishes ~2.1us
        # apart, so bias-add + output DMA overlap the matmul stream
        for ot in range(NT):
            for h in range(NH):
                for k in range(NK - NTAIL, NK):
                    for p in range(1, 5):
                        nc.tensor.matmul(
                            ps[(ot, h)],
                            cpts[(k, p)][:, ot * 128:(ot + 1) * 128],
                            pows[k][(p, h)][:],
                            start=False,
                            stop=(k == NK - 1 and p == 4),
                        )
                # bias-add split across both engines, halves DMA'd separately
                o_sb = opool.tile([128, 512], F32, tag="o_sb", name=f"o_{ot}_{h}")
                nc.scalar.activation(
                    o_sb[:, 0:256],
                    ps[(ot, h)][:, 0:256],
                    mybir.ActivationFunctionType.Identity,
                    bias=biascol[:, ot:ot + 1],
                )
                nc.vector.tensor_scalar_add(
                    o_sb[:, 256:512], ps[(ot, h)][:, 256:512], biascol[:, ot:ot + 1]
                )
                nc.sync.dma_start(
                    yt[ot * 128:(ot + 1) * 128, h * 512:h * 512 + 256],
                    o_sb[:, 0:256],
                )
                nc.sync.dma_start(
                    yt[ot * 128:(ot + 1) * 128, h * 512 + 256:(h + 1) * 512],
                    o_sb[:, 256:512],
                )

    nc.compile()
    return nc


def _get_nc():
    if "nc" not in _CACHE:
        _CACHE["nc"] = _build()
    return _CACHE["nc"]


def _make_in_maps(x, coeffs, bias):
    x = np.asarray(x, dtype=np.float32)
    coeffs = np.asarray(coeffs, dtype=np.float32)
    bias = np.asarray(bias, dtype=np.float32)

    xts = [
        np.ascontiguousarray(x[bg * BS:(bg + 1) * BS, :].T) for bg in range(BW)
    ]
    cts = [
        np.ascontiguousarray(
            coeffs[og * OS:(og + 1) * OS, :, 1:].transpose(2, 1, 0)
        )
        for og in range(OW)
    ]
    c0os = [
        np.ascontiguousarray(coeffs[og * OS:(og + 1) * OS, :, 0])
        for og in range(OW)
    ]
    in_maps = []
    for c in range(BW * OW):
        bg, og = c // OW, c % OW
        in_maps.append(
            {
                "xt": xts[bg],
                "ct": cts[og],
                "c0o": c0os[og],
                "biasc": np.ascontiguousarray(
                    bias[0, og * OS:(og + 1) * OS].reshape(OS, 1)
                ),
            }
        )
    return in_maps


def _gather(results):
    y = np.empty((B, O), dtype=np.float32)
    for c, res in enumerate(results):
        bg, og = c // OW, c % OW
        y[bg * BS:(bg + 1) * BS, og * OS:(og + 1) * OS] = res["yt"].T
    return y


def run(x, coeffs, bias, trace=False, **trace_kwargs):
    nc = _get_nc()
    in_maps = _make_in_maps(x, coeffs, bias)
    br = run_bass_kernel_spmd(
        nc, in_maps, list(range(BW * OW)), trace=trace, **trace_kwargs
    )
    return _gather(br.results), br


def kernel(x, coeffs, bias):
    out, _ = run(x, coeffs, bias)
    return out



# revision 3
# speedup vs baseline: 1.0332x; 1.0332x over previous
"""Trainium2 Bass kernel for KANPolyLayer:
    y[b,o] = sum_{i,p} x[b,i]^p * coeffs[o,i,p] + bias[o],  p = 0..4

Math: y = sum_{p=1..4} (x^p) @ C_p^T + biascol, where C_p = coeffs[:, :, p]
and biascol[o] = bias[o] + sum_i coeffs[o,i,0] is folded on the host.

All four GEMM planes run in bf16 (inputs rounded to bf16, fp32 PSUM
accumulation): same PE streaming rate as fp32r on TRN2, but enables
fast-weight-load (FWL) so LDWEIGHTS is fully hidden, and halves the
coefficient DMA stream.  Powers are computed on-chip from the fp32 x
(ScalarE squares, VectorE mul/copies) and rounded once to bf16.

Per-core schedule: bf16 power slabs ([i, b] layout) are built per
k-tile and stay resident; coefficient tiles stream through a ring.
All 8 output groups (4 o-tiles x 2 b-halves) accumulate concurrently
in 8 PSUM banks for k=0..5, then the last 2 k-tiles run group-major so
the 8 groups finish staggered ~1.8us apart: each group's bias-add
evacuation and output DMA overlaps the remaining matmul stream.
Input loads issue from the sync-engine DMA queue; output stores issue
from the vector-engine queue so they never queue behind input traffic.

Sharding (8 cores): 4 batch groups x 2 out-dim groups.
  core c -> (bg, og) = (c // 2, c % 2)
Each core computes a disjoint (512 x 1024) block of yT; host gathers.
"""

from contextlib import ExitStack

import numpy as np
import ml_dtypes

import concourse.bacc as bacc
import concourse.bass as bass
import concourse.mybir as mybir
import concourse.tile as tile
from concourse.bass_utils import run_bass_kernel_spmd

F32 = mybir.dt.float32
BF16 = mybir.dt.bfloat16

B, I, O = 4096, 1024, 1024  # batch, in_dim, out_dim
BW, OW = 4, 2               # batch groups x out-dim groups (8 cores)
BS, OS = B // BW, O // OW   # per-core batch (1024) and out (512)
NK = I // 128               # contraction tiles (8)
NT = OS // 128              # o-tiles (4)
NH = BS // 512              # b-halves (2)
NTAIL = 2                   # trailing k-planes emitted group-contiguous
NWARM = 16                  # PE warmup matmuls (N=256, cold ~213ns each)

_CACHE: dict = {}


def _build():
    nc = bacc.Bacc("TRN2", target_bir_lowering=False, debug=False, num_devices=8)

    xt = nc.dram_tensor("xt", [I, BS], F32, kind="ExternalInput")       # [i, b]
    ct = nc.dram_tensor("ct", [4, I, OS], BF16, kind="ExternalInput")   # [p-1, i, o]
    biasc = nc.dram_tensor("biasc", [OS, 1], F32, kind="ExternalInput")
    yt = nc.dram_tensor("yt", [OS, BS], F32, kind="ExternalOutput")     # [o, b]

    with tile.TileContext(nc) as tc, ExitStack() as ctx:
        cons = ctx.enter_context(tc.tile_pool(name="cons", bufs=1))
        cpool = ctx.enter_context(tc.tile_pool(name="coef", bufs=16))
        xpool = ctx.enter_context(tc.tile_pool(name="xin", bufs=6))
        ppool = ctx.enter_context(tc.tile_pool(name="pow", bufs=1))
        opool = ctx.enter_context(tc.tile_pool(name="out", bufs=1))
        pspool = ctx.enter_context(
            tc.tile_pool(name="ps", bufs=8, space=bass.MemorySpace.PSUM)
        )

        # 8 concurrent accumulation groups: (o-tile, b-half) -> one PSUM bank
        ps = {}
        for ot in range(NT):
            for h in range(NH):
                ps[(ot, h)] = pspool.tile(
                    [128, 512], F32, tag="ps", name=f"ps_{ot}_{h}"
                )

        # bias column: biascol[o-part, ot]; issued from the vector queue so
        # it doesn't delay the input stream on the sync queue.
        biascol = cons.tile([128, NT], F32)
        for ot in range(NT):
            nc.scalar.dma_start(
                biascol[:, ot:ot + 1], biasc[ot * 128:(ot + 1) * 128, :]
            )

        # PE warmup: garbage matmuls on a memset tile while the first input
        # DMAs are in flight, so the HAM clock-gate reaches 2.4 GHz before
        # the real stream starts.
        wz = cons.tile([128, 512], F32)
        nc.vector.memset(wz[:], 0.0)
        wr = cons.tile([128, 512], BF16)
        nc.vector.tensor_copy(wr[:], wz[:])
        for w in range(NWARM):
            nc.tensor.matmul(
                ps[(0, 0)][:, 0:256], wr[:, 0:128], wr[:, 0:256],
                start=True, stop=True, skip_group_check=True,
            )

        pows = {}   # (k, p, h) -> bf16 [128, 512] tile
        cpts = {}   # (k, p) -> bf16 [128, OS] tile
        for k in range(NK):
            tail_k = k >= NK - NTAIL
            # k0: first coefficient plane first so MM#1's weights land first
            if k == 0:
                cpt = cpool.tile([128, OS], BF16, tag="cp", name="cpt_0_1")
                nc.sync.dma_start(cpt[:], ct[0, 0:128, :])
                cpts[(0, 1)] = cpt
            for h2 in range(NH):
                x1 = xpool.tile([128, 512], F32, tag="x1", name=f"x1_{k}_{h2}")
                nc.sync.dma_start(
                    x1[:],
                    xt[k * 128:(k + 1) * 128, h2 * 512:(h2 + 1) * 512],
                )
                p1b = ppool.tile([128, 512], BF16, tag=f"p1_{k}_{h2}",
                                 name=f"p1_{k}_{h2}")
                p2b = ppool.tile([128, 512], BF16, tag=f"p2_{k}_{h2}",
                                 name=f"p2_{k}_{h2}")
                p3b = ppool.tile([128, 512], BF16, tag=f"p3_{k}_{h2}",
                                 name=f"p3_{k}_{h2}")
                p4b = ppool.tile([128, 512], BF16, tag=f"p4_{k}_{h2}",
                                 name=f"p4_{k}_{h2}")
                p2f = xpool.tile([128, 512], F32, tag="p2f", name=f"p2f_{k}_{h2}")
                nc.vector.tensor_copy(p1b[:], x1[:])
                nc.scalar.square(p2f[:], x1[:])
                nc.vector.tensor_copy(p2b[:], p2f[:])
                nc.vector.tensor_mul(p3b[:], p2f[:], x1[:])
                nc.scalar.square(p4b[:], p2f[:])
                pows[(k, 1, h2)] = p1b
                pows[(k, 2, h2)] = p2b
                pows[(k, 3, h2)] = p3b
                pows[(k, 4, h2)] = p4b

            for p in range(1, 5):
                if (k, p) not in cpts:
                    cpt = cpool.tile([128, OS], BF16, tag="cp", name=f"cpt_{k}_{p}")
                    nc.sync.dma_start(cpt[:], ct[p - 1, k * 128:(k + 1) * 128, :])
                    cpts[(k, p)] = cpt
                if not tail_k:
                    for ot in range(NT):
                        for h in range(NH):
                            nc.tensor.matmul(
                                ps[(ot, h)],
                                cpts[(k, p)][:, ot * 128:(ot + 1) * 128],
                                pows[(k, p, h)][:],
                                start=(k == 0 and p == 1),
                                stop=False,
                            )

        # trailing k-planes group-contiguous: groups finish ~1.8us apart, so
        # bias-add + output DMA overlap the remaining matmul stream
        for ot in range(NT):
            for h in range(NH):
                for k in range(NK - NTAIL, NK):
                    for p in range(1, 5):
                        nc.tensor.matmul(
                            ps[(ot, h)],
                            cpts[(k, p)][:, ot * 128:(ot + 1) * 128],
                            pows[(k, p, h)][:],
                            start=False,
                            stop=(k == NK - 1 and p == 4),
                        )
                # bias-add split across both engines, then one 256KB store
                # from the vector DMA queue (separate from input traffic)
                o_sb = opool.tile([128, 512], F32, tag=f"o_{ot}_{h}",
                                  name=f"o_{ot}_{h}")
                nc.scalar.activation(
                    o_sb[:, 0:256],
                    ps[(ot, h)][:, 0:256],
                    mybir.ActivationFunctionType.Identity,
                    bias=biascol[:, ot:ot + 1],
                )
                nc.vector.tensor_scalar_add(
                    o_sb[:, 256:512], ps[(ot, h)][:, 256:512], biascol[:, ot:ot + 1]
                )
                nc.scalar.dma_start(
                    yt[ot * 128:(ot + 1) * 128, h * 512:(h + 1) * 512],
                    o_sb[:],
                )

    nc.compile()
    return nc


def _get_nc():
    if "nc" not in _CACHE:
        _CACHE["nc"] = _build()
    return _CACHE["nc"]


def _make_in_maps(x, coeffs, bias):
    x = np.asarray(x, dtype=np.float32)
    coeffs = np.asarray(coeffs, dtype=np.float32)
    bias = np.asarray(bias, dtype=np.float32)

    xts = [
        np.ascontiguousarray(x[bg * BS:(bg + 1) * BS, :].T) for bg in range(BW)
    ]
    cts = [
        np.ascontiguousarray(
            coeffs[og * OS:(og + 1) * OS, :, 1:].transpose(2, 1, 0)
        ).astype(ml_dtypes.bfloat16)
        for og in range(OW)
    ]
    # biascol[o] = bias[o] + sum_i coeffs[o, i, 0]  (p=0 plane + bias)
    biascol = bias[0] + coeffs[:, :, 0].sum(axis=1)
    in_maps = []
    for c in range(BW * OW):
        bg, og = c // OW, c % OW
        in_maps.append(
            {
                "xt": xts[bg],
                "ct": cts[og],
                "biasc": np.ascontiguousarray(
                    biascol[og * OS:(og + 1) * OS].reshape(OS, 1)
                ).astype(np.float32),
            }
        )
    return in_maps


def _gather(results):
    y = np.empty((B, O), dtype=np.float32)
    for c, res in enumerate(results):
        bg, og = c // OW, c % OW
        y[bg * BS:(bg + 1) * BS, og * OS:(og + 1) * OS] = res["yt"].T
    return y


def run(x, coeffs, bias, trace=False, **trace_kwargs):
    nc = _get_nc()
    in_maps = _make_in_maps(x, coeffs, bias)
    br = run_bass_kernel_spmd(
        nc, in_maps, list(range(BW * OW)), trace=trace, **trace_kwargs
    )
    return _gather(br.results), br


def kernel(x, coeffs, bias):
    out, _ = run(x, coeffs, bias)
    return out


# revision 4
# speedup vs baseline: 1.2094x; 1.1706x over previous
"""Trainium2 Bass kernel for KANPolyLayer:
    y[b,o] = sum_{i,p} x[b,i]^p * coeffs[o,i,p] + bias[o],  p = 0..4

Math: y = sum_{p=1..4} (x^p) @ C_p^T + biascol, where C_p = coeffs[:, :, p]
and biascol[o] = bias[o] + sum_i coeffs[o,i,0] is folded on the host.

Precision split (rel-err budget 2e-2, measured 7.4e-3 on the real inputs):
  planes p=1,2  -> fp8e4 operands, fused into ONE DoubleRow matmul per
                   (k-tile, o-tile, half): 2 fp8 weights/PE cell, so both
                   planes stream in ~1.13x the time of one bf16 plane.
  planes p=3,4  -> bf16 operands (full PE rate + fast-weight-load).
All coefficients are pre-scaled by 512 on the host so the fp8 values sit
in e4m3's normal range; PSUM accumulates 512*y in fp32 and the evacuation
applies the 1/512 descale together with the bias column.

Per-core schedule: power slabs ([i, b] layout) are built per k-tile from
the fp32 x (ScalarE squares, VectorE mul/copies) and stay resident;
coefficient tiles stream through rings.  All 8 output groups (4 o-tiles x
2 b-halves) accumulate concurrently in 8 PSUM banks for k=0..5, then the
last 2 k-tiles run group-major so groups finish staggered: each group's
descale+bias evacuation and output DMA overlap the remaining matmul
stream.  Input loads issue from the sync-engine DMA queue; output stores
issue from the scalar-engine queue so they never sit behind input traffic.

Sharding (8 cores): 4 batch groups x 2 out-dim groups.
  core c -> (bg, og) = (c // 2, c % 2)
Each core computes a disjoint (512 x 1024) block of yT; host gathers.
"""

from contextlib import ExitStack

import numpy as np
import ml_dtypes

import concourse.bacc as bacc
import concourse.bass as bass
import concourse.mybir as mybir
import concourse.tile as tile
from concourse.bass_utils import run_bass_kernel_spmd

F32 = mybir.dt.float32
BF16 = mybir.dt.bfloat16
F8 = mybir.dt.float8e4

B, I, O = 4096, 1024, 1024  # batch, in_dim, out_dim
BW, OW = 4, 2               # batch groups x out-dim groups (8 cores)
BS, OS = B // BW, O // OW   # per-core batch (1024) and out (512)
NK = I // 128               # contraction tiles (8)
NT = OS // 128              # o-tiles (4)
NH = BS // 512              # b-halves (2)
NTAIL = 2                   # trailing k-planes emitted group-contiguous
NWARM = 4                   # fp32 warmup matmuls (N=256 cold ~850ns each)
CSCALE = 512.0              # host coefficient scale (fp8 range placement)

_CACHE: dict = {}


def _build():
    nc = bacc.Bacc("TRN2", target_bir_lowering=False, debug=False, num_devices=8)

    xt = nc.dram_tensor("xt", [I, BS], F32, kind="ExternalInput")        # [i, b]
    ct12 = nc.dram_tensor("ct12", [I, 2, OS], F8, kind="ExternalInput")  # fp8 p1,p2
    ct34 = nc.dram_tensor("ct34", [2, I, OS], BF16, kind="ExternalInput")
    biasc = nc.dram_tensor("biasc", [OS, 1], F32, kind="ExternalInput")
    yt = nc.dram_tensor("yt", [OS, BS], F32, kind="ExternalOutput")      # [o, b]

    DR = mybir.MatmulPerfMode.DoubleRow

    with tile.TileContext(nc) as tc, ExitStack() as ctx:
        cons = ctx.enter_context(tc.tile_pool(name="cons", bufs=1))
        c12pool = ctx.enter_context(tc.tile_pool(name="c12", bufs=8))
        c34pool = ctx.enter_context(tc.tile_pool(name="c34", bufs=12))
        xpool = ctx.enter_context(tc.tile_pool(name="xin", bufs=5))
        ppool = ctx.enter_context(tc.tile_pool(name="pow", bufs=1))
        opool = ctx.enter_context(tc.tile_pool(name="out", bufs=1))
        pspool = ctx.enter_context(
            tc.tile_pool(name="ps", bufs=8, space=bass.MemorySpace.PSUM)
        )

        # 8 concurrent accumulation groups: (o-tile, b-half) -> one PSUM bank
        ps = {}
        for ot in range(NT):
            for h in range(NH):
                ps[(ot, h)] = pspool.tile(
                    [128, 512], F32, tag="ps", name=f"ps_{ot}_{h}"
                )

        # bias column biascol[o-part, ot]: tiny loads on the scalar queue so
        # they don't delay the input stream on the sync queue.
        biascol = cons.tile([128, NT], F32)
        for ot in range(NT):
            nc.scalar.dma_start(
                biascol[:, ot:ot + 1], biasc[ot * 128:(ot + 1) * 128, :]
            )

        # PE warmup: fp32 garbage matmuls (1/4 rate -> long busy per inst,
        # no conversion dependency) so the HAM clock-gate reaches 2.4 GHz
        # before the real stream starts.
        wz = cons.tile([128, 512], F32)
        nc.gpsimd.memset(wz[:], 0.0)
        for w in range(NWARM):
            nc.tensor.matmul(
                ps[(0, 0)][:, 0:256], wz[:, 0:128], wz[:, 0:256],
                start=True, stop=True, skip_group_check=True,
            )

        pows = {}   # (k, 'dr'|3|4, h) -> power tile
        cp12 = {}   # k -> fp8 [128, 2, OS] tile
        cp34 = {}   # (k, p) -> bf16 [128, OS] tile
        for k in range(NK):
            tail_k = k >= NK - NTAIL
            # k0: DR coefficient tile first so the first matmul's weights
            # land first
            if k == 0:
                c = c12pool.tile([128, 2, OS], F8, tag="c12", name="cp12_0")
                nc.sync.dma_start(c[:], ct12[0:128, :, :])
                cp12[0] = c
            for h2 in range(NH):
                x1 = xpool.tile([128, 512], F32, tag="x1", name=f"x1_{k}_{h2}")
                nc.sync.dma_start(
                    x1[:],
                    xt[k * 128:(k + 1) * 128, h2 * 512:(h2 + 1) * 512],
                )
                p12q = ppool.tile([128, 2, 512], F8, tag=f"p12_{k}_{h2}",
                                  name=f"p12_{k}_{h2}")
                p3b = ppool.tile([128, 512], BF16, tag=f"p3_{k}_{h2}",
                                 name=f"p3_{k}_{h2}")
                p4b = ppool.tile([128, 512], BF16, tag=f"p4_{k}_{h2}",
                                 name=f"p4_{k}_{h2}")
                p2f = xpool.tile([128, 512], F32, tag="p2f", name=f"p2f_{k}_{h2}")
                nc.scalar.square(p2f[:], x1[:])           # x^2 fp32
                nc.vector.tensor_copy(p12q[:, 0, :], x1[:])   # x -> fp8
                nc.vector.tensor_copy(p12q[:, 1, :], p2f[:])  # x^2 -> fp8
                nc.vector.tensor_mul(p3b[:], p2f[:], x1[:])   # x^3 -> bf16
                nc.scalar.square(p4b[:], p2f[:])              # x^4 -> bf16
                pows[(k, 'dr', h2)] = p12q
                pows[(k, 3, h2)] = p3b
                pows[(k, 4, h2)] = p4b

            if k not in cp12:
                c = c12pool.tile([128, 2, OS], F8, tag="c12", name=f"cp12_{k}")
                nc.sync.dma_start(c[:], ct12[k * 128:(k + 1) * 128, :, :])
                cp12[k] = c
            for p in (3, 4):
                c = c34pool.tile([128, OS], BF16, tag="c34", name=f"cp34_{k}_{p}")
                nc.sync.dma_start(c[:], ct34[p - 3, k * 128:(k + 1) * 128, :])
                cp34[(k, p)] = c

            if not tail_k:
                for ot in range(NT):
                    for h in range(NH):
                        nc.tensor.matmul(
                            ps[(ot, h)],
                            cp12[k][:, :, ot * 128:(ot + 1) * 128],
                            pows[(k, 'dr', h)][:, :, :],
                            start=(k == 0),
                            stop=False,
                            perf_mode=DR,
                        )
                for p in (3, 4):
                    for ot in range(NT):
                        for h in range(NH):
                            nc.tensor.matmul(
                                ps[(ot, h)],
                                cp34[(k, p)][:, ot * 128:(ot + 1) * 128],
                                pows[(k, p, h)][:],
                                start=False,
                                stop=False,
                            )

        # trailing k-planes group-contiguous: groups finish staggered, so
        # descale+bias evacuation and output DMA overlap the matmul stream
        inv = 1.0 / CSCALE
        for ot in range(NT):
            for h in range(NH):
                for k in range(NK - NTAIL, NK):
                    nc.tensor.matmul(
                        ps[(ot, h)],
                        cp12[k][:, :, ot * 128:(ot + 1) * 128],
                        pows[(k, 'dr', h)][:, :, :],
                        start=False,
                        stop=False,
                        perf_mode=DR,
                    )
                    for p in (3, 4):
                        nc.tensor.matmul(
                            ps[(ot, h)],
                            cp34[(k, p)][:, ot * 128:(ot + 1) * 128],
                            pows[(k, p, h)][:],
                            start=False,
                            stop=(k == NK - 1 and p == 4),
                        )
                # descale + bias-add split across both engines, then one
                # 256KB store from the scalar DMA queue
                o_sb = opool.tile([128, 512], F32, tag=f"o_{ot}_{h}",
                                  name=f"o_{ot}_{h}")
                nc.scalar.activation(
                    o_sb[:, 0:256],
                    ps[(ot, h)][:, 0:256],
                    mybir.ActivationFunctionType.Identity,
                    bias=biascol[:, ot:ot + 1],
                    scale=inv,
                )
                nc.vector.tensor_scalar(
                    o_sb[:, 256:512],
                    ps[(ot, h)][:, 256:512],
                    inv,
                    biascol[:, ot:ot + 1],
                    mybir.AluOpType.mult,
                    mybir.AluOpType.add,
                )
                nc.scalar.dma_start(
                    yt[ot * 128:(ot + 1) * 128, h * 512:(h + 1) * 512],
                    o_sb[:],
                )

    nc.compile()
    return nc


def _get_nc():
    if "nc" not in _CACHE:
        _CACHE["nc"] = _build()
    return _CACHE["nc"]


def _make_in_maps(x, coeffs, bias):
    x = np.asarray(x, dtype=np.float32)
    coeffs = np.asarray(coeffs, dtype=np.float32)
    bias = np.asarray(bias, dtype=np.float32)

    xts = [
        np.ascontiguousarray(x[bg * BS:(bg + 1) * BS, :].T) for bg in range(BW)
    ]
    c12s = [
        np.ascontiguousarray(
            (coeffs[og * OS:(og + 1) * OS, :, 1:3] * CSCALE).transpose(1, 2, 0)
        ).astype(ml_dtypes.float8_e4m3)
        for og in range(OW)
    ]
    c34s = [
        np.ascontiguousarray(
            (coeffs[og * OS:(og + 1) * OS, :, 3:5] * CSCALE).transpose(2, 1, 0)
        ).astype(ml_dtypes.bfloat16)
        for og in range(OW)
    ]
    # biascol[o] = bias[o] + sum_i coeffs[o, i, 0]  (p=0 plane + bias)
    biascol = bias[0] + coeffs[:, :, 0].sum(axis=1)
    in_maps = []
    for c in range(BW * OW):
        bg, og = c // OW, c % OW
        in_maps.append(
            {
                "xt": xts[bg],
                "ct12": c12s[og],
                "ct34": c34s[og],
                "biasc": np.ascontiguousarray(
                    biascol[og * OS:(og + 1) * OS].reshape(OS, 1)
                ).astype(np.float32),
            }
        )
    return in_maps


def _gather(results):
    y = np.empty((B, O), dtype=np.float32)
    for c, res in enumerate(results):
        bg, og = c // OW, c % OW
        y[bg * BS:(bg + 1) * BS, og * OS:(og + 1) * OS] = res["yt"].T
    return y


def run(x, coeffs, bias, trace=False, **trace_kwargs):
    nc = _get_nc()
    in_maps = _make_in_maps(x, coeffs, bias)
    br = run_bass_kernel_spmd(
        nc, in_maps, list(range(BW * OW)), trace=trace, **trace_kwargs
    )
    return _gather(br.results), br


def kernel(x, coeffs, bias):
    out, _ = run(x, coeffs, bias)
    return out
